# revision 1
# baseline (speedup 1.0000x reference)
"""Trainium2 Bass kernel for nn_Binder_MiniGrid (topk_masking).

Algebraic reduction: with q == bq constant, attention scores collapse to a
single linear functional of the LN'd conv features, folded into two extra
output columns of the conv matmul (s1, mu). Only the conv runs dense
(451 GFLOP total); k/v projections are never materialized.

Conv as matmul: the input is host-packed into 41-wide valid-position rows
per (jb, j) shift — partition j*32+c holds x[c, jb*4+j + r*48 + ow] at
free index jb*2009 + r*41 + ow — so a contiguous [128, 123] slice is
exactly 3 image rows x 41 valid positions (stationary operand, M=123).
14 position tiles x 16 (kh, jb) contraction chunks per sample (vs 16 for
the naive 48-wide flattening: 12.5% fewer streamed PE columns), fp32r at
1 col/cycle with N=258 >= 256 keeping the per-matmul LDWEIGHTS hidden.
fp32r (12-bit mantissa) is required: top-4 score margins go down to
~6e-6 while bf16 conv noise is ~4e-4 — bf16 flips selections.

Top-4 selection stays 2D: per-partition Max8 on [123, 14] scores, one
984-element gather, then a global Max8 gives the exact 4th-largest
(kth). Scores are LN-normalized (|s| <= ~1) so the softmax exp needs no
max-subtraction — selected-exp weights exp(s)*(s >= kth) compute with
full 123-lane parallelism; kth broadcasts to all partitions via a K=1
matmul and 1/sumw folds into the pooled-row drain (ACT scale operand).
Junk positions (tile 13 rows beyond the image) are capped to -1e9 via
one stt-min against a host mask. Pooling: DVE weighted accumulation
(two interleaved chains), then one M=1 matmul (ones^T @ acc, 1-column
LDWEIGHTS, N=258 f32r) per sample into a psum row, drained to a flat
channel-major buffer; one transpose via 2 contiguous DMAs at the end
feeds the final wv projection + LN2 + ReLU. The previous sample's pool
chain is interleaved step-by-step into the current sample's conv tile
drains (smooths the DVE queue so psum drains never stall the PE), and
the score chain runs one sample behind the conv stream.

Data parallel over 8 NeuronCores: 32 of the 256 stacked (curr; targ)
samples per core. TimelineSim: 824 us/core (v1 baseline: 957); measured
end-to-end error vs the fp32 reference ~2.5e-4.
"""
import sys
sys.path.insert(0, "/opt/trn_rl_repo")
import numpy as np
import ml_dtypes

import concourse.bacc as bacc
import concourse.tile as tile
from concourse import mybir
from concourse.bass_types import AP as BassAP
from concourse.bass_utils import run_bass_kernel_spmd

F32 = mybir.dt.float32
F32R = mybir.dt.float32r
BF16 = mybir.dt.bfloat16
AF = mybir.ActivationFunctionType
ALU = mybir.AluOpType

B, C, HH, WW = 128, 32, 48, 48
E, KF, TOPK, EPS = 256, 8, 4, 1e-5
OH = 41
L = OH * OH            # 1681
NS = 32                # samples per core
N_CORES = 8
NT = 14                # position tiles per sample: 13x123 + (82 valid of 123)
ROWL = 123 * NT        # 1722
XW4 = 49 * 41          # 2009: packed window-start cols per jb half
XWP = 2352             # host row width: max read jb*4+j+47*48+40 = 2351
NE = E + 2             # 258: conv channels + s1 + mu columns
NEG = -1.0e9

_CACHE = {}
_DT = {"np": np.float32}


def _round_f32r(a):
    bits = np.ascontiguousarray(a, np.float32).view(np.uint32).astype(np.uint64)
    r = ((bits + 0x800) & ~np.uint64(0xFFF)).astype(np.uint32)
    return r.view(np.float32).reshape(a.shape)


def build_nc(ns=NS, scheme="v14", dtype="f32r", bufs=4, psum_bufs=None):
    dt_in = F32R if dtype == "f32r" else BF16
    _DT["np"] = np.float32 if dtype == "f32r" else ml_dtypes.bfloat16
    _DT["name"] = dtype
    nc = bacc.Bacc()
    x_t = nc.dram_tensor("x", [ns, 128, 2 * XW4], dt_in, kind="ExternalInput")
    warr_ap = nc.dram_tensor("warr", [128, 16 * NE], dt_in, kind="ExternalInput").ap()
    brd_ap = nc.dram_tensor("brd", [128, NE], F32, kind="ExternalInput").ap()
    wvg_ap = nc.dram_tensor("wvg", [128, 2 * E], F32, kind="ExternalInput").ap()
    cc_ap = nc.dram_tensor("cc", [2, E], F32, kind="ExternalInput").ap()
    g2b2_ap = nc.dram_tensor("g2b2", [NS, 2 * E], F32, kind="ExternalInput").ap()
    onesr_ap = nc.dram_tensor("onesr", [128, 1], F32R, kind="ExternalInput").ap()
    capm_ap = nc.dram_tensor("capm", [128, NT], F32, kind="ExternalInput").ap()
    out_ap = nc.dram_tensor("out", [ns, E], F32, kind="ExternalOutput").ap()

    with tile.TileContext(nc) as tc:
        with tc.tile_pool(name="const", bufs=1) as cpool, \
             tc.tile_pool(name="samp", bufs=bufs) as sp, \
             tc.tile_pool(name="xin", bufs=3) as spx, \
             tc.tile_pool(name="small", bufs=3) as smp, \
             tc.tile_pool(name="rows", bufs=2) as rp, \
             tc.tile_pool(name="psum", bufs=psum_bufs or 5, space="PSUM") as pp, \
             tc.tile_pool(name="psum_row", bufs=1, space="PSUM") as prp:

            # ---- static loads
            w_sb = cpool.tile([128, 16 * NE], dt_in)
            nc.sync.dma_start(w_sb[:], warr_ap[:])
            brd = cpool.tile([128, NE], F32)
            nc.sync.dma_start(brd[:], brd_ap[:])
            wvg = cpool.tile([128, 2 * E], F32)
            nc.sync.dma_start(wvg[:], wvg_ap[:])
            cc = cpool.tile([2, E], F32)
            nc.sync.dma_start(cc[:], cc_ap[:])
            g2b2 = cpool.tile([NS, 2 * E], F32)
            nc.sync.dma_start(g2b2[:], g2b2_ap[:])
            textra = cpool.tile([2, NS], F32)
            nc.vector.memset(textra[0:2, :], 1.0)  # row 0 overwritten by T DMA below
            c_eeps = cpool.tile([128, 1], F32)
            nc.vector.memset(c_eeps[:], float(E) * EPS)
            c_eps = cpool.tile([NS, 1], F32)
            nc.vector.memset(c_eps[:], EPS)
            ones_col = cpool.tile([128, 1], F32R)
            nc.sync.dma_start(ones_col[:], onesr_ap[:])
            capm = cpool.tile([128, NT], F32)
            nc.sync.dma_start(capm[:], capm_ap[:])
            ones_row = cpool.tile([1, 128], F32)
            nc.vector.memset(ones_row[:], 1.0)

            # flat pooled rows: partition 0, channel-major [e*ns + s]
            pflat = cpool.tile([1, NE * ns], F32)
            pflat_v = pflat[0:1, :].rearrange("p (e s) -> p e s", s=ns)

            def emit_conv(x4, f_s, ss, pool_step=None):
                def slab(r0, jb):
                    # [128, 123] = 3 packed 41-wide rows starting at row r0
                    return x4[:, jb * XW4 + r0 * 41: jb * XW4 + r0 * 41 + 123]

                def drain(t, pt):
                    ft = f_s[:, t * NE:(t + 1) * NE]
                    nc.vector.tensor_add(ft[:123, :], pt[:123, :], brd[:123, :])
                    sq = smp.tile([128, E], F32, tag="sq")
                    nc.scalar.activation(sq[:123, :], ft[:123, 0:E], AF.Square,
                                         accum_out=ss[:123, t:t + 1])

                if scheme == "v14":
                    for t in range(NT):
                        pt = pp.tile([128, NE], F32)
                        for ki in range(16):
                            kh, jb = ki // 2, ki % 2
                            nc.tensor.matmul(pt[:123, :], slab(3 * t + kh, jb),
                                             w_sb[:, ki * NE:(ki + 1) * NE],
                                             start=(ki == 0), stop=(ki == 15),
                                             skip_group_check=True)
                        drain(t, pt)
                        if pool_step is not None:
                            pool_step(t)
                elif scheme == "r0":
                    pts = {}
                    for r0 in range(47):
                        t_lo = max(0, (r0 - 5) // 3)
                        t_hi = min(13, r0 // 3)
                        for jb in range(2):
                            for t in range(t_lo, t_hi + 1):
                                kh = r0 - 3 * t
                                ki = kh * 2 + jb
                                if kh == 0 and jb == 0:
                                    pts[t] = pp.tile([128, NE], F32, name="pt",
                                                     tag="pt")
                                nc.tensor.matmul(pts[t][:123, :], slab(r0, jb),
                                                 w_sb[:, ki * NE:(ki + 1) * NE],
                                                 start=(kh == 0 and jb == 0),
                                                 stop=(kh == 7 and jb == 1),
                                                 skip_group_check=True)
                        if r0 >= 7 and (r0 - 7) % 3 == 0:
                            t = (r0 - 7) // 3
                            drain(t, pts.pop(t))
                    for t in sorted(pts):
                        drain(t, pts.pop(t))
                else:
                    raise ValueError(scheme)

            def make_pool_stepper(si, fsi, wsi, reci):
                st = {}

                def step(t):
                    if t == 0:
                        st["acc2"] = smp.tile([128, NE], F32, tag="acc2", name="acc2")
                        st["acc"] = smp.tile([128, NE], F32R, tag="acc", name="acc")
                        nc.vector.tensor_scalar(st["acc2"][:123, :],
                                                fsi[:123, 0:NE],
                                                wsi[:123, 0:1], None,
                                                op0=ALU.mult)
                        nc.vector.tensor_scalar(st["acc"][:123, :],
                                                fsi[:123, NE:2 * NE],
                                                wsi[:123, 1:2], None,
                                                op0=ALU.mult)
                    elif t <= NT - 2:
                        u = t + 1
                        dst = st["acc2"] if u % 2 == 0 else st["acc"]
                        nc.vector.scalar_tensor_tensor(
                            dst[:123, :], fsi[:123, u * NE:(u + 1) * NE],
                            wsi[:123, u:u + 1], dst[:123, :],
                            op0=ALU.mult, op1=ALU.add)
                    else:
                        nc.vector.tensor_add(st["acc"][:123, :],
                                             st["acc"][:123, :],
                                             st["acc2"][:123, :])
                        prow = prp.tile([1, NE], F32, name="prow",
                                        tag="prow", bufs=2)
                        nc.tensor.matmul(prow[0:1, :], ones_col[:123, 0:1],
                                         st["acc"][:123, :], start=True,
                                         stop=True, skip_group_check=True)
                        nc.scalar.activation(pflat_v[0:1, :, si], prow[0:1, :],
                                             AF.Copy, scale=reci[0:1, 0:1])
                return step

            prev = None
            for s in range(ns):
                # ---- X4: host-packed [j*32+c, jb*2009 + r*41 + ow], one DMA
                x4 = spx.tile([128, 2 * XW4], dt_in)
                chunks = ((0, 10), (10, 20), (20, 30), (30, 40), (40, 49)) \
                    if s == 0 else ((0, 16), (16, 32), (32, 49))
                for jb in range(2):
                    for a, b in chunks:
                        off = jb * XW4 + a * 41
                        w = (b - a) * 41
                        src = BassAP(x_t, s * 128 * 2 * XW4 + off,
                                     [[2 * XW4, 128], [1, w]])
                        nc.sync.dma_start(x4[:, off:off + w], src)

                f_s = sp.tile([128, NT * NE], F32)
                ss = smp.tile([128, NT], F32)   # sum of squares per position
                stepper = make_pool_stepper(*prev) if prev is not None else None
                emit_conv(x4, f_s, ss, stepper)

                # ---- scores on (123, NT)
                fv = f_s[:, :].rearrange("p (t e) -> p t e", t=NT)
                mu_v = fv[:123, :, E + 1]
                s1_v = fv[:123, :, E]
                sc1 = smp.tile([128, NT], F32, tag="sc1")
                nc.vector.tensor_mul(sc1[:123, :], mu_v, mu_v)
                nc.vector.scalar_tensor_tensor(sc1[:123, :], sc1[:123, :],
                                               -float(E), ss[:123, :],
                                               op0=ALU.mult, op1=ALU.add)
                sig = smp.tile([128, NT], F32, tag="sig")
                nc.scalar.activation(sig[:123, :], sc1[:123, :], AF.Sqrt,
                                     bias=c_eeps[:123, 0:1])
                inv2 = smp.tile([128, NT], F32, tag="inv2")
                nc.vector.reciprocal(inv2[:123, :], sig[:123, :])
                # host folded "- S*mu" into the s1 column: score = s1_v * inv2
                scs = smp.tile([128, NT], F32, tag="scs")
                nc.vector.tensor_mul(scs[:123, :], s1_v, inv2[:123, :])
                nc.vector.scalar_tensor_tensor(scs[:123, :], scs[:123, :], 1.0,
                                               capm[:123, :], op0=ALU.mult,
                                               op1=ALU.min)

                # ---- top-k: per-partition top-8, then global top-8 of
                # the 984 candidates. Scores are LN-normalized (|s| <= ~1),
                # so exp needs no max-subtraction; the selected-exp weights
                # compute 2D on [123, NT] with full lane parallelism. kth
                # broadcasts to all partitions via a K=1 matmul; 1/sumw is
                # folded into the pooled-row drain.
                m8p = smp.tile([128, 8], F32, tag="m8p")
                nc.vector.max(out=m8p[:123, :], in_=scs[:123, :])
                row = rp.tile([1, 984], F32, tag="row")
                nc.sync.dma_start(row[0:1, :], m8p[:123, :])
                m8 = smp.tile([1, 8], F32, tag="m8")
                nc.vector.max(out=m8[:], in_=row[:])
                e4 = smp.tile([1, 4], F32, tag="e4")
                sumw = smp.tile([1, 1], F32, tag="sumw")
                nc.scalar.activation(e4[:], m8[:, 0:4], AF.Exp,
                                     accum_out=sumw[:])
                rec = smp.tile([1, 1], F32, tag="rec")
                nc.vector.reciprocal(rec[:], sumw[:])
                kbc_ps = prp.tile([128, 1], F32, name="kbc", tag="kbc", bufs=1)
                nc.tensor.matmul(kbc_ps[0:123, :], ones_row[0:1, 0:123],
                                 m8[0:1, 3:4], start=True, stop=True,
                                 skip_group_check=True)
                kbc = smp.tile([128, 1], F32, tag="kbc_sb")
                nc.scalar.copy(kbc[0:123, :], kbc_ps[0:123, :])

                # ---- selected-exp weights on (123, NT), * inv2
                erow2 = smp.tile([128, NT], F32, tag="erow2")
                nc.scalar.activation(erow2[:123, :], scs[:123, :], AF.Exp)
                w_s = smp.tile([128, NT], F32, tag="w_s")
                nc.vector.scalar_tensor_tensor(w_s[:123, :], scs[:123, :],
                                               kbc[:123, 0:1], erow2[:123, :],
                                               op0=ALU.is_ge, op1=ALU.mult)
                nc.vector.tensor_mul(w_s[:123, :], w_s[:123, :], inv2[:123, :])

                # (pool chain of the previous sample is interleaved into
                # this sample's conv via make_pool_stepper)
                prev = (s, f_s, w_s, rec)

            # last sample's pool chain runs as a block after the conv loop
            stepper = make_pool_stepper(*prev)
            for t in range(NT):
                stepper(t)

            # ---- tail: transpose pooled rows -> [e, s], projection + LN2 + relu
            psbT = cpool.tile([128, 2 * ns], F32)
            nc.sync.dma_start(psbT[:, 0:ns], pflat[0:1, 0:128 * ns])
            nc.sync.dma_start(psbT[:, ns:2 * ns], pflat[0:1, 128 * ns:256 * ns])
            nc.sync.dma_start(textra[0:1, 0:ns],
                              pflat[0:1, (E + 1) * ns:(E + 2) * ns])
            outp = prp.tile([NS, E], F32, tag="kbc", bufs=1)
            nc.tensor.matmul(outp[0:ns, :], psbT[:, 0:ns], wvg[:, 0:E],
                             start=True, stop=False, skip_group_check=True)
            nc.tensor.matmul(outp[0:ns, :], psbT[:, ns:2 * ns], wvg[:, E:2 * E],
                             start=False, stop=False, skip_group_check=True)
            nc.tensor.matmul(outp[0:ns, :], textra[:, 0:ns], cc[:],
                             start=False, stop=True, skip_group_check=True)

            srow = cpool.tile([NS, 1], F32)
            nc.vector.reduce_sum(srow[0:ns, :], outp[0:ns, :],
                                 axis=mybir.AxisListType.X)
            nmu = cpool.tile([NS, 1], F32)
            nc.vector.tensor_scalar(nmu[0:ns, :], srow[0:ns, :], -1.0 / E, None,
                                    op0=ALU.mult)
            cent = cpool.tile([NS, E], F32)
            nc.vector.tensor_scalar(cent[0:ns, :], outp[0:ns, :],
                                    nmu[0:ns, 0:1], None, op0=ALU.add)
            sq2 = cpool.tile([NS, E], F32)
            ssq = cpool.tile([NS, 1], F32)
            nc.scalar.activation(sq2[0:ns, :], cent[0:ns, :], AF.Square,
                                 accum_out=ssq[0:ns, :])
            sigf = cpool.tile([NS, 1], F32)
            nc.scalar.activation(sigf[0:ns, :], ssq[0:ns, :], AF.Sqrt,
                                 bias=c_eps[0:ns, 0:1], scale=1.0 / E)
            invf = cpool.tile([NS, 1], F32)
            nc.vector.reciprocal(invf[0:ns, :], sigf[0:ns, :])
            nc.vector.scalar_tensor_tensor(cent[0:ns, :], cent[0:ns, :],
                                           invf[0:ns, 0:1], g2b2[0:ns, 0:E],
                                           op0=ALU.mult, op1=ALU.mult)
            nc.vector.tensor_add(cent[0:ns, :], cent[0:ns, :], g2b2[0:ns, E:2 * E])
            yout = cpool.tile([NS, E], F32)
            nc.scalar.activation(yout[0:ns, :], cent[0:ns, :], AF.Relu)
            nc.sync.dma_start(out_ap[:], yout[0:ns, :])

    nc.finalize()
    return nc


def host_prep(inputs, ns=NS):
    """Build per-core input maps + the stacked sample array."""
    conv_w = np.asarray(inputs["conv_w"], np.float32)
    conv_b = np.asarray(inputs["conv_b"], np.float32)
    g1 = np.asarray(inputs["ln1_g"], np.float32)
    b1 = np.asarray(inputs["ln1_b"], np.float32)
    wk = np.asarray(inputs["wk"], np.float32)
    bk = np.asarray(inputs["bk"], np.float32)
    bq = np.asarray(inputs["bq"], np.float32)
    wv = np.asarray(inputs["wv"], np.float32)
    bv = np.asarray(inputs["bv"], np.float32)
    g2 = np.asarray(inputs["ln2_g"], np.float32)
    b2 = np.asarray(inputs["ln2_b"], np.float32)

    W2 = conv_w.transpose(1, 2, 3, 0).reshape(C * KF * KF, E)  # [(c,kh,kw), e]
    u = wk.T @ bq
    ug = u * g1
    S = float(ug.sum())
    col_mu = W2 @ (np.ones(E, np.float32) / E)
    col_s1 = W2 @ ug - S * col_mu
    W_aug = np.concatenate([W2, col_s1[:, None], col_mu[:, None]], 1)
    b_mu = float(conv_b.mean())
    b_s1 = float(conv_b @ ug) - S * b_mu
    bias_aug = np.concatenate([conv_b, [b_s1], [b_mu]]).astype(np.float32)

    # rearrange W_aug into the 16 (kh, jb) chunks, k = j*32 + c
    warr = np.zeros((128, 16 * NE), np.float32)
    for ki in range(16):
        kh, jb = ki // 2, ki % 2
        for c in range(C):
            for j in range(4):
                warr[j * 32 + c, ki * NE:(ki + 1) * NE] = \
                    W_aug[c * 64 + kh * 8 + jb * 4 + j]

    brd = np.tile(bias_aug[None, :], (128, 1)).astype(np.float32)

    sqE = np.sqrt(np.float32(E))
    wvg_m = (wv * g1[None, :]) * sqE          # (f, e)
    wvgT = wvg_m.T                            # (e, f)
    wvg = np.zeros((128, 2 * E), np.float32)
    wvg[:, 0:E] = wvgT[0:128, :]
    wvg[:, E:2 * E] = wvgT[128:256, :]

    vec_t = wvg_m.sum(axis=1)                 # sqE * sum_e wv*g1
    const_vec = wv @ b1 + bv
    cc = np.stack([-vec_t, const_vec]).astype(np.float32)

    g2b2 = np.zeros((NS, 2 * E), np.float32)
    g2b2[:, 0:E] = g2[None, :]
    g2b2[:, E:2 * E] = b2[None, :]

    capm = np.full((128, NT), 3.0e38, np.float32)
    capm[82:123, NT - 1] = NEG

    x_all = np.concatenate([np.asarray(inputs["state_curr"], np.float32),
                            np.asarray(inputs["state_targ"], np.float32)], 0)
    n_total = x_all.shape[0]
    xf = x_all.reshape(n_total, C, HH * WW)
    xp = np.zeros((n_total, C, XWP), np.float32)
    xp[:, :, 0:HH * WW] = xf

    # pack: xpk[n, j*32+c, jb*2009 + r*41 + ow] = xp[n, c, jb*4+j + r*48 + ow]
    from numpy.lib.stride_tricks import as_strided
    xpk = np.empty((n_total, 128, 2 * XW4), np.float32)
    sn, sc, se = xp.strides
    for jb in range(2):
        for j in range(4):
            view = as_strided(xp[:, :, jb * 4 + j:],
                              shape=(n_total, C, 49, 41),
                              strides=(sn, sc, 48 * se, se))
            xpk[:, j * 32:(j + 1) * 32,
                jb * XW4:(jb + 1) * XW4] = view.reshape(n_total, C, XW4)

    npdt = _DT["np"]
    if npdt is np.float32:
        warr_c = _round_f32r(warr)
        xp_c = _round_f32r(xpk)
    else:
        warr_c = warr.astype(npdt)
        xp_c = xpk.astype(npdt)

    n_cores = n_total // ns
    in_maps = []
    for i in range(n_cores):
        in_maps.append({
            "x": xp_c[i * ns:(i + 1) * ns],
            "warr": warr_c, "brd": brd, "wvg": wvg, "cc": cc, "g2b2": g2b2,
            "onesr": np.ones((128, 1), np.float32), "capm": capm,
        })
    return in_maps


def kernel(state_curr, state_targ, conv_w, conv_b, ln1_g, ln1_b,
           wq, bq, wk, bk, wv, bv, ln2_g, ln2_b):
    inputs = dict(state_curr=state_curr, state_targ=state_targ,
                  conv_w=conv_w, conv_b=conv_b, ln1_g=ln1_g, ln1_b=ln1_b,
                  wq=wq, bq=bq, wk=wk, bk=bk, wv=wv, bv=bv,
                  ln2_g=ln2_g, ln2_b=ln2_b)
    if "nc" not in _CACHE:
        _CACHE["nc"] = build_nc(NS)
    nc = _CACHE["nc"]
    in_maps = host_prep(inputs, NS)
    res = run_bass_kernel_spmd(nc, in_maps, list(range(N_CORES)), trace=False)
    outs = [res.results[i]["out"] for i in range(N_CORES)]
    full = np.concatenate(outs, 0)            # (256, 256)
    nb = state_curr.shape[0]
    return np.concatenate([full[:nb], full[nb:]], axis=-1).astype(np.float32)

